# revision 10
# baseline (speedup 1.0000x reference)
"""Trainium2 Bass kernel for nn_CrossMatrix.

Computes, per batch row b (B = 65536 rows total):
    x1   = [1, x[b]]                      (65,)
    y[j] = sum_{a,c} W[j, a*65+c] * x1[a] * x1[c] + bias[j]     (64,)
    out  = LayerNorm(y) * gamma + beta  -> reshape (8, 8)

Decomposition used on-device (A[j,a,c] = W[j, a*65+c]):
    y[b,j] = sum_{a,c>=1} A[j,a,c] x_a x_c        (quadratic part)
           + sum_{c>=1} (A[j,0,c] + A[j,c,0]) x_c (linear part)
           + A[j,0,0] + bias[j]                   (constant part)

  stage-1 (TensorE): Z[b, (j,a)] = sum_c x[b,c] * V[c, (j,a)],
      V[c, j*64+a] = A[j, a+1, c+1]   -> 8 matmuls of N=512 per 128-row tile
  lin    (TensorE): ylin[b, j] = sum_c x[b,c] * L[c, j] (+ const via bcast add)
  stage-2 (VectorE): y[b,j] = ylin + sum_a x[b,a] * Z[b, (j,a)]
  LayerNorm (VectorE/ScalarE): bn_stats/bn_aggr + rsqrt + tensor_scalar.

Sharding: pure data parallel, batch split across 8 cores (8192 rows each).
"""

import numpy as np

import concourse.bass as bass
from concourse import bacc
import concourse.mybir as mybir
import concourse.tile as tile
from concourse.bass_utils import run_bass_kernel_spmd
from concourse.masks import make_identity

# Problem constants (hardcoded per harness contract).
B = 65536
N = 64          # features out (8*8)
NX = 64         # features in (x per row)
INNER = 4225    # (N+1)**2
EPS = 1e-6
N_CORES = 8
ROWS_PER_CORE = B // N_CORES  # 8192
P = 128                       # partitions / batch tile rows

F32 = mybir.dt.float32
F32R = mybir.dt.float32r
BF16 = mybir.dt.bfloat16

_CACHE = {}


def _build_bass(rows_per_core, apply_gamma, apply_beta):
    ntiles = rows_per_core // P
    nc = bacc.Bacc(None, target_bir_lowering=False)

    xs = nc.declare_dram_parameter("xs", [rows_per_core, NX], F32, isOutput=False)
    vq = nc.declare_dram_parameter("vq", [NX, N * NX], BF16, isOutput=False)
    lw = nc.declare_dram_parameter("lw", [NX, N], BF16, isOutput=False)
    cb = nc.declare_dram_parameter("cb", [P, N], F32, isOutput=False)
    if apply_gamma:
        gm = nc.declare_dram_parameter("gm", [P, N], F32, isOutput=False)
    if apply_beta:
        bt = nc.declare_dram_parameter("bt", [P, N], F32, isOutput=False)
    out = nc.declare_dram_parameter("out", [rows_per_core, N], F32, isOutput=True)

    with (
        tile.TileContext(nc) as tc,
        tc.tile_pool(name="consts", bufs=1) as consts,
        tc.tile_pool(name="xpool", bufs=3) as xpool,
        tc.tile_pool(name="xtpool", bufs=2) as xtpool,
        tc.tile_pool(name="ppool", bufs=3) as ppool,
        tc.tile_pool(name="ypool", bufs=3) as ypool,
        tc.tile_pool(name="opool", bufs=3) as opool,
        tc.tile_pool(name="stat", bufs=4) as stat,
        tc.tile_pool(name="zpsum", bufs=3, space="PSUM") as zpsum,
        tc.tile_pool(name="ypsum", bufs=2, space="PSUM") as ypsum,
        tc.tile_pool(name="tpsum", bufs=1, space="PSUM") as tpsum,
        tc.tile_pool(name="scrps", bufs=1, space="PSUM") as scrps,
    ):
        # --- constants in SBUF ---
        ident = consts.tile([P, P], F32)
        make_identity(nc, ident)
        v_sb = consts.tile([NX, N * NX], BF16)
        nc.gpsimd.dma_start(out=v_sb, in_=vq[:, :])
        l_sb = consts.tile([NX, N], BF16)
        nc.gpsimd.dma_start(out=l_sb, in_=lw[:, :])
        c_sb = consts.tile([P, N], F32)
        nc.gpsimd.dma_start(out=c_sb, in_=cb[:, :])
        if apply_gamma:
            g_sb = consts.tile([P, N], F32)
            nc.gpsimd.dma_start(out=g_sb, in_=gm[:, :])
        if apply_beta:
            b_sb = consts.tile([P, N], F32)
            nc.gpsimd.dma_start(out=b_sb, in_=bt[:, :])
        eps_sb = consts.tile([P, 1], F32)
        nc.vector.memset(eps_sb, EPS)

        # --- warm-up absorbers ---
        # The PE weight-load slot only fits one semaphore wait, so make each
        # engine observe every const producer once, via single-dep ops, before
        # the steady-state loop (keeps every later instruction at <=1 fresh
        # cross-engine wait).
        scr_t = scrps.tile([P, P], F32, tag="scr")
        nc.tensor.transpose(scr_t, ident, ident)  # PE observes gpsimd memset
        scr_m = scrps.tile([1, 1], F32, tag="scr")
        nc.tensor.matmul(scr_m, lhsT=v_sb[:, 0:1], rhs=v_sb[:, 0:1],
                         start=True, stop=True)  # PE observes V dma
        scr_m2 = scrps.tile([1, 1], F32, tag="scr")
        nc.tensor.matmul(scr_m2, lhsT=l_sb[:, 0:1], rhs=l_sb[:, 0:1],
                         start=True, stop=True)  # PE observes L dma
        scr_v = consts.tile([P, 1], F32)
        nc.vector.tensor_copy(scr_v, c_sb[:, 0:1])  # DVE observes C dma
        if apply_gamma:
            nc.vector.tensor_copy(scr_v, g_sb[:, 0:1])
        if apply_beta:
            nc.vector.tensor_copy(scr_v, b_sb[:, 0:1])

        for t in range(ntiles):
            r0 = t * P
            # load x tile [128, 64]
            x_sb = xpool.tile([P, NX], F32)
            nc.gpsimd.dma_start(out=x_sb, in_=xs[r0 : r0 + P, :])

            # transpose -> xt [64, 128] (TensorE, lands in PSUM, evict to SBUF)
            xt_ps = tpsum.tile([NX, P], F32)
            nc.tensor.transpose(xt_ps, x_sb, ident)
            xt_sb = xtpool.tile([NX, P], BF16)
            nc.vector.tensor_copy(out=xt_sb, in_=xt_ps)

            # DVE-local copy of x so the stage-2 multiplies never wait on DMA
            xf_sb = xtpool.tile([P, NX], F32)
            nc.vector.tensor_copy(out=xf_sb, in_=x_sb)

            yq_sb = ypool.tile([P, N], F32)

            # stage-1 + stage-2 per 512-wide chunk (8 j-values x 64 a)
            for k in range(8):
                z_ps = zpsum.tile([P, 512], F32)
                nc.tensor.matmul(
                    z_ps,
                    lhsT=xt_sb[:, :],
                    rhs=v_sb[:, k * 512 : (k + 1) * 512],
                    start=True,
                    stop=True,
                )
                # P = Z * x (broadcast x over the 8 j's in this chunk)
                p_sb = ppool.tile([P, 8, NX], F32)
                x_bc = xf_sb[:, :].unsqueeze(1).broadcast_to((P, 8, NX))
                nc.vector.tensor_mul(
                    p_sb,
                    z_ps[:, :].rearrange("p (j a) -> p j a", a=NX),
                    x_bc,
                )
                # y_quad[:, 8k:8k+8] = sum_a P
                nc.vector.reduce_sum(
                    out=yq_sb[:, k * 8 : (k + 1) * 8],
                    in_=p_sb,
                    axis=mybir.AxisListType.X,
                )

            # linear part on TensorE: ylin = xT.T @ L  -> [128, 64]
            yl_ps = ypsum.tile([P, N], F32)
            nc.tensor.matmul(
                yl_ps,
                lhsT=xt_sb[:, :],
                rhs=l_sb[:, :],
                start=True,
                stop=True,
            )

            # y = yq + ylin + const
            y_sb = ypool.tile([P, N], F32)
            nc.vector.tensor_add(y_sb, yq_sb, yl_ps)
            nc.vector.tensor_add(y_sb, y_sb, c_sb)

            # LayerNorm over the 64 features
            st_sb = stat.tile([P, 6], F32)
            nc.vector.bn_stats(out=st_sb, in_=y_sb)
            mv_sb = stat.tile([P, 2], F32)
            nc.vector.bn_aggr(out=mv_sb, in_=st_sb)
            # rstd = 1/sqrt(var + eps)
            sd_sb = stat.tile([P, 1], F32)
            nc.scalar.activation(
                out=sd_sb,
                in_=mv_sb[:, 1:2],
                func=mybir.ActivationFunctionType.Sqrt,
                bias=eps_sb,
                scale=1.0,
            )
            rs_sb = stat.tile([P, 1], F32)
            nc.vector.reciprocal(out=rs_sb, in_=sd_sb)

            o_sb = opool.tile([P, N], F32)
            nc.vector.tensor_scalar(
                out=o_sb,
                in0=y_sb,
                scalar1=mv_sb[:, 0:1],
                scalar2=rs_sb,
                op0=mybir.AluOpType.subtract,
                op1=mybir.AluOpType.mult,
            )
            if apply_gamma:
                nc.vector.tensor_mul(o_sb, o_sb, g_sb)
            if apply_beta:
                nc.vector.tensor_add(o_sb, o_sb, b_sb)

            nc.gpsimd.dma_start(out=out[r0 : r0 + P, :], in_=o_sb)

    if not nc.is_finalized():
        nc.finalize()
    return nc


def _get_bass(rows_per_core, apply_gamma, apply_beta):
    key = (rows_per_core, apply_gamma, apply_beta)
    if key not in _CACHE:
        _CACHE[key] = _build_bass(rows_per_core, apply_gamma, apply_beta)
    return _CACHE[key]


def _host_constants(W, b, gamma, beta):
    A = np.asarray(W, np.float32).reshape(N, NX + 1, NX + 1)  # [j, a, c]
    # V[c, j*64 + a] = A[j, a+1, c+1]
    import ml_dtypes
    V = np.ascontiguousarray(
        A[:, 1:, 1:].transpose(2, 0, 1).reshape(NX, N * NX)
    ).astype(ml_dtypes.bfloat16)
    # L[c, j] = A[j, 0, c+1] + A[j, c+1, 0]
    L = np.ascontiguousarray((A[:, 0, 1:] + A[:, 1:, 0]).T).astype(ml_dtypes.bfloat16)
    # const per j (+ linear bias), broadcast to all 128 partitions
    cvec = A[:, 0, 0] + np.asarray(b, np.float32)
    C = np.ascontiguousarray(np.tile(cvec[None, :], (P, 1))).astype(np.float32)
    G = np.ascontiguousarray(np.tile(np.asarray(gamma, np.float32)[None, :], (P, 1)))
    Bt = np.ascontiguousarray(np.tile(np.asarray(beta, np.float32)[None, :], (P, 1)))
    return V, L, C, G, Bt


def kernel(x, W, b, gamma, beta, _rows_per_core=ROWS_PER_CORE, _trace=False):
    x = np.ascontiguousarray(np.asarray(x, np.float32))
    V, L, C, G, Bt = _host_constants(W, b, gamma, beta)
    apply_gamma = not np.all(np.asarray(gamma) == 1.0)
    apply_beta = not np.all(np.asarray(beta) == 0.0)

    nc = _get_bass(_rows_per_core, apply_gamma, apply_beta)

    in_maps = []
    for c in range(N_CORES):
        m = {
            "xs": np.ascontiguousarray(
                x[c * _rows_per_core : (c + 1) * _rows_per_core, :]
            ),
            "vq": V,
            "lw": L,
            "cb": C,
        }
        if apply_gamma:
            m["gm"] = G
        if apply_beta:
            m["bt"] = Bt
        in_maps.append(m)

    res = run_bass_kernel_spmd(
        nc, in_maps, core_ids=list(range(N_CORES)), trace=_trace
    )
    outs = [res.results[i]["out"] for i in range(N_CORES)]
    full = np.concatenate(outs, axis=0)  # [8 * rows_per_core, 64]
    if _trace:
        kernel._last_result = res
    return full.reshape(-1, 8, 8).astype(np.float32)


# revision 16
# speedup vs baseline: 1.9926x; 1.9926x over previous
"""Trainium2 Bass kernel for nn_CrossMatrix.

Computes, per batch row b (B = 65536 rows total):
    x1   = [1, x[b]]                      (65,)
    y[j] = sum_{a,c} W[j, a*65+c] * x1[a] * x1[c] + bias[j]     (64,)
    out  = LayerNorm(y) * gamma + beta  -> reshape (8, 8)

Decomposition used on-device (A[j,a,c] = W[j, a*65+c]):
    y[b,j] = sum_{a,c>=1} A[j,a,c] x_a x_c        (quadratic part)
           + sum_{c>=1} (A[j,0,c] + A[j,c,0]) x_c (linear part)
           + A[j,0,0] + bias[j]                   (constant part)

  stage-1 (TensorE): Z[b, (j,a)] = sum_c x[b,c] * V[c, (j,a)],
      V[c, j*64+a] = A[j, a+1, c+1]   -> 8 matmuls of N=512 per 128-row tile
  lin    (TensorE): ylin[b, j] = sum_c x[b,c] * L[c, j] (+ const via bcast add)
  stage-2 (VectorE): y[b,j] = ylin + sum_a x[b,a] * Z[b, (j,a)]
  LayerNorm (VectorE/ScalarE): bn_stats/bn_aggr + rsqrt + tensor_scalar.

Sharding: pure data parallel, batch split across 8 cores (8192 rows each).
"""

import numpy as np

import concourse.bass as bass
from concourse import bacc
import concourse.mybir as mybir
import concourse.tile as tile
from concourse.bass_utils import run_bass_kernel_spmd
from concourse.masks import make_identity

# Problem constants (hardcoded per harness contract).
B = 65536
N = 64          # features out (8*8)
NX = 64         # features in (x per row)
INNER = 4225    # (N+1)**2
EPS = 1e-6
N_CORES = 8
ROWS_PER_CORE = B // N_CORES  # 8192
P = 128                       # partitions / batch tile rows

F32 = mybir.dt.float32
F32R = mybir.dt.float32r
BF16 = mybir.dt.bfloat16
FP16 = mybir.dt.float16
V3DT = FP16  # dtype for the whole v3 PE datapath
G2DT = FP16  # dtype for squared projections + selection matrix

_CACHE = {}


def _build_bass(rows_per_core, apply_gamma, apply_beta):
    ntiles = rows_per_core // P
    nc = bacc.Bacc(None, target_bir_lowering=False)

    xs = nc.declare_dram_parameter("xs", [rows_per_core, NX], F32, isOutput=False)
    vq = nc.declare_dram_parameter("vq", [NX, N * NX], BF16, isOutput=False)
    lw = nc.declare_dram_parameter("lw", [NX, N], BF16, isOutput=False)
    cb = nc.declare_dram_parameter("cb", [P, N], F32, isOutput=False)
    if apply_gamma:
        gm = nc.declare_dram_parameter("gm", [P, N], F32, isOutput=False)
    if apply_beta:
        bt = nc.declare_dram_parameter("bt", [P, N], F32, isOutput=False)
    out = nc.declare_dram_parameter("out", [rows_per_core, N], F32, isOutput=True)

    with (
        tile.TileContext(nc) as tc,
        tc.tile_pool(name="consts", bufs=1) as consts,
        tc.tile_pool(name="xpool", bufs=3) as xpool,
        tc.tile_pool(name="xtpool", bufs=2) as xtpool,
        tc.tile_pool(name="ppool", bufs=3) as ppool,
        tc.tile_pool(name="ypool", bufs=3) as ypool,
        tc.tile_pool(name="opool", bufs=3) as opool,
        tc.tile_pool(name="stat", bufs=4) as stat,
        tc.tile_pool(name="zpsum", bufs=3, space="PSUM") as zpsum,
        tc.tile_pool(name="ypsum", bufs=2, space="PSUM") as ypsum,
        tc.tile_pool(name="tpsum", bufs=1, space="PSUM") as tpsum,
        tc.tile_pool(name="scrps", bufs=1, space="PSUM") as scrps,
    ):
        # --- constants in SBUF ---
        ident = consts.tile([P, P], F32)
        make_identity(nc, ident)
        v_sb = consts.tile([NX, N * NX], BF16)
        nc.gpsimd.dma_start(out=v_sb, in_=vq[:, :])
        l_sb = consts.tile([NX, N], BF16)
        nc.gpsimd.dma_start(out=l_sb, in_=lw[:, :])
        c_sb = consts.tile([P, N], F32)
        nc.gpsimd.dma_start(out=c_sb, in_=cb[:, :])
        if apply_gamma:
            g_sb = consts.tile([P, N], F32)
            nc.gpsimd.dma_start(out=g_sb, in_=gm[:, :])
        if apply_beta:
            b_sb = consts.tile([P, N], F32)
            nc.gpsimd.dma_start(out=b_sb, in_=bt[:, :])
        eps_sb = consts.tile([P, 1], F32)
        nc.vector.memset(eps_sb, EPS)

        # --- warm-up absorbers ---
        # The PE weight-load slot only fits one semaphore wait, so make each
        # engine observe every const producer once, via single-dep ops, before
        # the steady-state loop (keeps every later instruction at <=1 fresh
        # cross-engine wait).
        scr_t = scrps.tile([P, P], F32, tag="scr")
        nc.tensor.transpose(scr_t, ident, ident)  # PE observes gpsimd memset
        scr_m = scrps.tile([1, 1], F32, tag="scr")
        nc.tensor.matmul(scr_m, lhsT=v_sb[:, 0:1], rhs=v_sb[:, 0:1],
                         start=True, stop=True)  # PE observes V dma
        scr_m2 = scrps.tile([1, 1], F32, tag="scr")
        nc.tensor.matmul(scr_m2, lhsT=l_sb[:, 0:1], rhs=l_sb[:, 0:1],
                         start=True, stop=True)  # PE observes L dma
        scr_v = consts.tile([P, 1], F32)
        nc.vector.tensor_copy(scr_v, c_sb[:, 0:1])  # DVE observes C dma
        if apply_gamma:
            nc.vector.tensor_copy(scr_v, g_sb[:, 0:1])
        if apply_beta:
            nc.vector.tensor_copy(scr_v, b_sb[:, 0:1])

        for t in range(ntiles):
            r0 = t * P
            # load x tile [128, 64]
            x_sb = xpool.tile([P, NX], F32)
            nc.gpsimd.dma_start(out=x_sb, in_=xs[r0 : r0 + P, :])

            # transpose -> xt [64, 128] (TensorE, lands in PSUM, evict to SBUF)
            xt_ps = tpsum.tile([NX, P], F32)
            nc.tensor.transpose(xt_ps, x_sb, ident)
            xt_sb = xtpool.tile([NX, P], BF16)
            nc.vector.tensor_copy(out=xt_sb, in_=xt_ps)

            # DVE-local copy of x so the stage-2 multiplies never wait on DMA
            xf_sb = xtpool.tile([P, NX], F32)
            nc.vector.tensor_copy(out=xf_sb, in_=x_sb)

            yq_sb = ypool.tile([P, N], F32)

            # stage-1 + stage-2 per 512-wide chunk (8 j-values x 64 a)
            for k in range(8):
                z_ps = zpsum.tile([P, 512], F32)
                nc.tensor.matmul(
                    z_ps,
                    lhsT=xt_sb[:, :],
                    rhs=v_sb[:, k * 512 : (k + 1) * 512],
                    start=True,
                    stop=True,
                )
                # P = Z * x (broadcast x over the 8 j's in this chunk)
                p_sb = ppool.tile([P, 8, NX], F32)
                x_bc = xf_sb[:, :].unsqueeze(1).broadcast_to((P, 8, NX))
                nc.vector.tensor_mul(
                    p_sb,
                    z_ps[:, :].rearrange("p (j a) -> p j a", a=NX),
                    x_bc,
                )
                # y_quad[:, 8k:8k+8] = sum_a P
                nc.vector.reduce_sum(
                    out=yq_sb[:, k * 8 : (k + 1) * 8],
                    in_=p_sb,
                    axis=mybir.AxisListType.X,
                )

            # linear part on TensorE: ylin = xT.T @ L  -> [128, 64]
            yl_ps = ypsum.tile([P, N], F32)
            nc.tensor.matmul(
                yl_ps,
                lhsT=xt_sb[:, :],
                rhs=l_sb[:, :],
                start=True,
                stop=True,
            )

            # y = yq + ylin + const
            y_sb = ypool.tile([P, N], F32)
            nc.vector.tensor_add(y_sb, yq_sb, yl_ps)
            nc.vector.tensor_add(y_sb, y_sb, c_sb)

            # LayerNorm over the 64 features
            st_sb = stat.tile([P, 6], F32)
            nc.vector.bn_stats(out=st_sb, in_=y_sb)
            mv_sb = stat.tile([P, 2], F32)
            nc.vector.bn_aggr(out=mv_sb, in_=st_sb)
            # rstd = 1/sqrt(var + eps)
            sd_sb = stat.tile([P, 1], F32)
            nc.scalar.activation(
                out=sd_sb,
                in_=mv_sb[:, 1:2],
                func=mybir.ActivationFunctionType.Sqrt,
                bias=eps_sb,
                scale=1.0,
            )
            rs_sb = stat.tile([P, 1], F32)
            nc.vector.reciprocal(out=rs_sb, in_=sd_sb)

            o_sb = opool.tile([P, N], F32)
            nc.vector.tensor_scalar(
                out=o_sb,
                in0=y_sb,
                scalar1=mv_sb[:, 0:1],
                scalar2=rs_sb,
                op0=mybir.AluOpType.subtract,
                op1=mybir.AluOpType.mult,
            )
            if apply_gamma:
                nc.vector.tensor_mul(o_sb, o_sb, g_sb)
            if apply_beta:
                nc.vector.tensor_add(o_sb, o_sb, b_sb)

            nc.gpsimd.dma_start(out=out[r0 : r0 + P, :], in_=o_sb)

    if not nc.is_finalized():
        nc.finalize()
    return nc


def _build_bass_v3(rows_per_core, apply_gamma, apply_beta):
    """Eigendecomposition formulation, feature-major.

    Host factors each symmetrized quadratic form: y_quad_j = sum_k s_jk*(u'_jk.x)^2
    with u' = sqrt(|lambda|)-scaled eigenvectors. On device, per 512-row block:
      G^T chunks [128 (j,k) rows, 512 batch] = U2_chunk.T @ XT   (TensorE)
      square G^T during PSUM->SBUF eviction                      (ScalarE/VectorE)
      y^T += R_chunk.T @ (G^T)^2  (signed selection, PSUM accum) (TensorE)
      y^T += L65.T @ XT65         (linear + const via ones row)  (TensorE)
      transpose y^T back to batch-major (TensorE) + LayerNorm    (VectorE/ScalarE)
    This keeps the per-element work (the squares) on ScalarE/VectorE at one pass
    over G, and does both contractions AND the k-reduction on TensorE.
    """
    ntiles = rows_per_core // P
    nblocks = rows_per_core // 512
    NK = N * NX  # 4096 (j,k) pairs
    nc = bacc.Bacc(None, target_bir_lowering=False)

    xs = nc.declare_dram_parameter("xs", [rows_per_core, NX], F32, isOutput=False)
    u2 = nc.declare_dram_parameter("u2", [NX, NK], V3DT, isOutput=False)
    rsel = nc.declare_dram_parameter("rsel", [P, 32 * N], G2DT, isOutput=False)
    l65 = nc.declare_dram_parameter("l65", [NX + 1, N], V3DT, isOutput=False)
    if apply_gamma:
        gm = nc.declare_dram_parameter("gm", [P, N], F32, isOutput=False)
    if apply_beta:
        bt = nc.declare_dram_parameter("bt", [P, N], F32, isOutput=False)
    out = nc.declare_dram_parameter("out", [rows_per_core, N], F32, isOutput=True)

    with (
        tile.TileContext(nc) as tc,
        tc.tile_pool(name="consts", bufs=1) as consts,
        tc.tile_pool(name="xpool", bufs=4) as xpool,
        tc.tile_pool(name="g2pool", bufs=4) as g2pool,
        tc.tile_pool(name="ysb", bufs=2) as ysb,
        tc.tile_pool(name="opool", bufs=4) as opool,
        tc.tile_pool(name="stat", bufs=8) as stat,
        tc.tile_pool(name="gps", bufs=2, space="PSUM") as gps,
        tc.tile_pool(name="ytps", bufs=2, space="PSUM") as ytps,
        tc.tile_pool(name="tps", bufs=1, space="PSUM") as tps,
        tc.tile_pool(name="ybps", bufs=1, space="PSUM") as ybps,
    ):
        ident = consts.tile([P, P], F32)
        make_identity(nc, ident)
        u2_sb = consts.tile([NX, NK], V3DT)
        nc.sync.dma_start(out=u2_sb, in_=u2[:, :])
        rs_sb = consts.tile([P, 32 * N], G2DT)
        nc.sync.dma_start(out=rs_sb, in_=rsel[:, :])
        l65_sb = consts.tile([NX + 1, N], V3DT)
        nc.sync.dma_start(out=l65_sb, in_=l65[:, :])
        if apply_gamma:
            g_sb = consts.tile([P, N], F32)
            nc.sync.dma_start(out=g_sb, in_=gm[:, :])
        if apply_beta:
            b_sb = consts.tile([P, N], F32)
            nc.sync.dma_start(out=b_sb, in_=bt[:, :])
        eps_sb = consts.tile([P, 1], F32)
        nc.vector.memset(eps_sb, EPS)

        # transposed x for the whole core slice, with a trailing ones row
        xt65 = consts.tile([NX + 1, rows_per_core], V3DT)
        nc.vector.memset(xt65[NX : NX + 1, :], 1.0)

        # warm-up absorbers (keep fresh cross-engine waits at <=1 per inst)
        scr_t = gps.tile([P, P], F32, tag="g")
        nc.tensor.transpose(scr_t, ident, ident)
        scr_m = gps.tile([1, 1], F32, tag="g")
        nc.tensor.matmul(scr_m, lhsT=u2_sb[:, 0:1], rhs=u2_sb[:, 0:1],
                         start=True, stop=True)
        scr_m2 = gps.tile([1, 1], F32, tag="g")
        nc.tensor.matmul(scr_m2, lhsT=rs_sb[:, 0:1], rhs=rs_sb[:, 0:1],
                         start=True, stop=True)
        scr_m3 = gps.tile([1, 1], F32, tag="g")
        nc.tensor.matmul(scr_m3, lhsT=l65_sb[:, 0:1], rhs=l65_sb[:, 0:1],
                         start=True, stop=True)

        for t in range(ntiles):
            r0 = t * P
            x_sb = xpool.tile([P, NX], F32)
            nc.sync.dma_start(out=x_sb, in_=xs[r0 : r0 + P, :])
            xt_ps = tps.tile([NX, P], F32)
            nc.tensor.transpose(xt_ps, x_sb, ident)
            nc.vector.tensor_copy(out=xt65[0:NX, r0 : r0 + P], in_=xt_ps)

        # how many of the 16 chunk-pairs per block ScalarE squares directly
        # (the rest go VectorE: bf16 copy out of PSUM, then square in SBUF)
        ACT_PAIRS = 11

        for s in range(nblocks):
            c0 = s * 512
            yt_ps = ytps.tile([N, 512], F32)
            for pr in range(16):
                g_ps = gps.tile([P, 1024], F32, tag="g")
                for h in range(2):
                    c = 2 * pr + h
                    nc.tensor.matmul(
                        g_ps[:, h * 512 : (h + 1) * 512],
                        lhsT=u2_sb[:, c * P : (c + 1) * P],
                        rhs=xt65[0:NX, c0 : c0 + 512],
                        start=True,
                        stop=True,
                    )
                g2_sb = g2pool.tile([P, 1024], G2DT)
                if pr < ACT_PAIRS:
                    nc.scalar.square(g2_sb, g_ps)
                else:
                    gc_sb = g2pool.tile([P, 1024], G2DT, tag="gc")
                    nc.vector.tensor_copy(out=gc_sb, in_=g_ps)
                    nc.vector.tensor_mul(g2_sb, gc_sb, gc_sb)
                for h in range(2):
                    c = 2 * pr + h
                    nc.tensor.matmul(
                        yt_ps,
                        lhsT=rs_sb[:, c * N : (c + 1) * N],
                        rhs=g2_sb[:, h * 512 : (h + 1) * 512],
                        start=(c == 0),
                        stop=False,
                    )
            # linear + const part (ones row of xt65)
            nc.tensor.matmul(
                yt_ps,
                lhsT=l65_sb,
                rhs=xt65[:, c0 : c0 + 512],
                start=False,
                stop=True,
            )
            yt_sb = ysb.tile([N, 512], F32)
            nc.vector.tensor_copy(out=yt_sb, in_=yt_ps)

            for tt in range(4):
                yb_ps = ybps.tile([P, N], F32)
                nc.tensor.transpose(
                    yb_ps, yt_sb[:, tt * P : (tt + 1) * P], ident[0:N, 0:N]
                )
                st_sb = stat.tile([P, 6], F32)
                nc.vector.bn_stats(out=st_sb, in_=yb_ps)
                mv_sb = stat.tile([P, 2], F32)
                nc.vector.bn_aggr(out=mv_sb, in_=st_sb)
                sd_sb = stat.tile([P, 1], F32)
                nc.scalar.activation(
                    out=sd_sb,
                    in_=mv_sb[:, 1:2],
                    func=mybir.ActivationFunctionType.Sqrt,
                    bias=eps_sb,
                    scale=1.0,
                )
                rq_sb = stat.tile([P, 1], F32)
                nc.vector.reciprocal(out=rq_sb, in_=sd_sb)
                o_sb = opool.tile([P, N], F32)
                nc.vector.tensor_scalar(
                    out=o_sb,
                    in0=yb_ps,
                    scalar1=mv_sb[:, 0:1],
                    scalar2=rq_sb,
                    op0=mybir.AluOpType.subtract,
                    op1=mybir.AluOpType.mult,
                )
                if apply_gamma:
                    nc.vector.tensor_mul(o_sb, o_sb, g_sb)
                if apply_beta:
                    nc.vector.tensor_add(o_sb, o_sb, b_sb)
                r0 = c0 + tt * P
                nc.sync.dma_start(out=out[r0 : r0 + P, :], in_=o_sb)

    if not nc.is_finalized():
        nc.finalize()
    return nc


def _g2_is_bf16():
    return G2DT == BF16


def _host_constants_v3(W, b):
    import ml_dtypes

    A = np.asarray(W, np.float64).reshape(N, NX + 1, NX + 1)  # [j, a, c]
    Q = A[:, 1:, 1:]
    S = 0.5 * (Q + Q.transpose(0, 2, 1))
    lam, U = np.linalg.eigh(S)  # lam [j, k], U [j, c, k]
    U2 = U * np.sqrt(np.abs(lam))[:, None, :]  # [j, c, k]
    # u2[c, j*64 + k]
    u2 = np.ascontiguousarray(
        U2.transpose(1, 0, 2).reshape(NX, N * NX)
    ).astype(np.float16)
    # rsel chunks: rsel[:, t*64:(t+1)*64][r, j'] = sign(lam[j, k]) * (j == j')
    # where jk = t*128 + r, j = jk // 64, k = jk % 64
    sgn = np.sign(lam)  # [j, k]
    rsel = np.zeros((32, P, N), np.float32)
    jk = np.arange(N * NX)
    jj = jk // NX
    kk = jk % NX
    rsel[jk // P, jk % P, jj] = sgn[jj, kk]
    rsel = np.ascontiguousarray(rsel.transpose(1, 0, 2).reshape(P, 32 * N)).astype(
        np.float16
    )
    # l65: rows 0..63 linear weights, row 64 const (incl. bias)
    Lw = (A[:, 0, 1:] + A[:, 1:, 0]).T  # [c, j]
    cvec = A[:, 0, 0] + np.asarray(b, np.float64)
    l65c = np.concatenate([Lw, cvec[None, :]], axis=0).astype(np.float16)
    return u2, rsel, np.ascontiguousarray(l65c)


def _get_bass(rows_per_core, apply_gamma, apply_beta, version=3):
    key = (rows_per_core, apply_gamma, apply_beta, version)
    if key not in _CACHE:
        if version == 3:
            _CACHE[key] = _build_bass_v3(rows_per_core, apply_gamma, apply_beta)
        else:
            _CACHE[key] = _build_bass(rows_per_core, apply_gamma, apply_beta)
    return _CACHE[key]


def _host_constants(W, b, gamma, beta):
    A = np.asarray(W, np.float32).reshape(N, NX + 1, NX + 1)  # [j, a, c]
    # V[c, j*64 + a] = A[j, a+1, c+1]
    import ml_dtypes
    V = np.ascontiguousarray(
        A[:, 1:, 1:].transpose(2, 0, 1).reshape(NX, N * NX)
    ).astype(ml_dtypes.bfloat16)
    # L[c, j] = A[j, 0, c+1] + A[j, c+1, 0]
    L = np.ascontiguousarray((A[:, 0, 1:] + A[:, 1:, 0]).T).astype(ml_dtypes.bfloat16)
    # const per j (+ linear bias), broadcast to all 128 partitions
    cvec = A[:, 0, 0] + np.asarray(b, np.float32)
    C = np.ascontiguousarray(np.tile(cvec[None, :], (P, 1))).astype(np.float32)
    G = np.ascontiguousarray(np.tile(np.asarray(gamma, np.float32)[None, :], (P, 1)))
    Bt = np.ascontiguousarray(np.tile(np.asarray(beta, np.float32)[None, :], (P, 1)))
    return V, L, C, G, Bt


def _make_in_maps(x, W, b, gamma, beta, rows_per_core, version=3):
    x = np.ascontiguousarray(np.asarray(x, np.float32))
    apply_gamma = not np.all(np.asarray(gamma) == 1.0)
    apply_beta = not np.all(np.asarray(beta) == 0.0)
    G = np.ascontiguousarray(np.tile(np.asarray(gamma, np.float32)[None, :], (P, 1)))
    Bt = np.ascontiguousarray(np.tile(np.asarray(beta, np.float32)[None, :], (P, 1)))
    in_maps = []
    if version == 3:
        u2, rsel, l65c = _host_constants_v3(W, b)
        for c in range(N_CORES):
            m = {
                "xs": np.ascontiguousarray(
                    x[c * rows_per_core : (c + 1) * rows_per_core, :]
                ),
                "u2": u2,
                "rsel": rsel,
                "l65": l65c,
            }
            if apply_gamma:
                m["gm"] = G
            if apply_beta:
                m["bt"] = Bt
            in_maps.append(m)
    else:
        V, L, C, _, _ = _host_constants(W, b, gamma, beta)
        for c in range(N_CORES):
            m = {
                "xs": np.ascontiguousarray(
                    x[c * rows_per_core : (c + 1) * rows_per_core, :]
                ),
                "vq": V,
                "lw": L,
                "cb": C,
            }
            if apply_gamma:
                m["gm"] = G
            if apply_beta:
                m["bt"] = Bt
            in_maps.append(m)
    return in_maps, apply_gamma, apply_beta


def kernel(x, W, b, gamma, beta, _rows_per_core=ROWS_PER_CORE, _trace=False,
           _version=3):
    in_maps, apply_gamma, apply_beta = _make_in_maps(
        x, W, b, gamma, beta, _rows_per_core, _version
    )
    nc = _get_bass(_rows_per_core, apply_gamma, apply_beta, _version)
    res = run_bass_kernel_spmd(
        nc, in_maps, core_ids=list(range(N_CORES)), trace=_trace
    )
    outs = [res.results[i]["out"] for i in range(N_CORES)]
    full = np.concatenate(outs, axis=0)  # [8 * rows_per_core, 64]
    if _trace:
        kernel._last_result = res
    return full.reshape(-1, 8, 8).astype(np.float32)
